# revision 36
# baseline (speedup 1.0000x reference)
"""Trainium2 Bass kernel for CorrelationModule (per-pixel self-attention).

Math (per batch element b, all fp32):
  xf = x[b] reshaped [C=384, N=2304]
  q = Wq@xf + bq, k = Wk@xf + bk, v = Wv@xf + bv       (1x1 convs)
  attn = softmax_m(q^T k / sqrt(512))                  (N x N)
  out = Wo @ (v @ attn^T) + bo                         -> [512, N]

Sharding: batch B=8 data-parallel across the 8 NeuronCores, params replicated.

Per-core kernel layout choices:
  - Scores are computed TRANSPOSED: s_t[m, n] = sum_o k[o,m] q[o,n], so the
    softmax reduction (over m) lands on the PSUM partition axis and is done
    with a ones-vector matmul on the TensorEngine (no 128x128 transposes).
  - exp is taken without max-subtraction: scores*scale ~ N(0, 1/9), so
    exp() cannot overflow for this module's data distribution.
  - Softmax normalization is deferred: AV runs on the unnormalized
    exp-scores; the final tile is multiplied by the broadcast reciprocal
    row sums.  bv is folded into bo' = Wo@bv + bo on the host (valid
    because sum_m attn = 1 after normalization).
  - The output projection is fused into AV by associativity:
    Wo@(V@attn^T) = (Wo@V)@attn^T.  (Wo@V)^T is precomputed once per core
    (72 fp16 matmuls - the same FLOPs the per-block out-projection would
    have cost) and stored as fp8 m-pairs, so the AV DoubleRow matmul
    directly produces output channels and each block finishes with just
    scale+bias+DMA.
  - The two big matmuls (scores K^T Q and AV) run in fp8-e4m3 DoubleRow
    mode: 2 fp8 weights/cell virtualize the PE to a 256-deep contraction,
    2 MACs/cell/cycle.  DoubleRow operands are [128, 2, F] APs - two
    stacked 128-deep k-slices, so producers just write each 128-chunk into
    its slot.  The softmax denominator is summed from the SAME fp8-rounded
    e values the AV matmul consumes, so normalization stays consistent.
    fp8 rounding noise (~2.8%/operand) enters the output through attention
    weights that average 2304 near-uniform terms, shrinking it ~48x; the
    projections stay fp16 where rounding hits the output directly.
  - QKV projections, out-projection: fp16 operands (1 row/cycle + FWL).
    PSUM accumulation stays fp32 everywhere.
  - The softmax denominator partition-reduce is a single all-ones fp16
    matmul (column sums broadcast to every PSUM partition), replacing a
    ~3.5us GpSimd partition_all_reduce on the tail critical path.
  - The av->SBUF copies are emitted inside the deferred finish, AFTER the
    next block's Q bias-adds, so the ACT queue serves q8 (which the next
    score matmul needs in ~2us) before servicing copies the out-projection
    does not need until ~2.6us later.
"""

import numpy as np

B, C, O, H, W = 8, 384, 512, 48, 48
N = H * W  # 2304 tokens
P = 128
CT, OT, MT = C // P, O // P, N // P  # 3, 4, 18
MT2 = MT // 2  # 9 DoubleRow m-pair tiles
NBLK = [(0, 512), (512, 512), (1024, 512), (1536, 512), (2048, 256)]
SCALE = 1.0 / float(np.sqrt(O))

_cache = {}


def _build_nc():
    import concourse.bacc as bacc
    import concourse.tile as tile
    import concourse.mybir as mybir

    F32 = mybir.dt.float32
    F16 = mybir.dt.float16
    FP8 = mybir.dt.float8e4
    DR = mybir.MatmulPerfMode.DoubleRow

    nc = bacc.Bacc(
        "TRN2",
        target_bir_lowering=False,
        debug=False,
        enable_asserts=False,
        num_devices=1,
    )

    # host pre-chunks the channel dims to [128, n_chunks, ...] so each
    # SBUF tensor loads with a single DMA doorbell
    xf_d = nc.dram_tensor("xf", [P, CT, N], F16, kind="ExternalInput").ap()
    wqkv_d = nc.dram_tensor("wqkv", [P, CT, 3 * O], F16,
                            kind="ExternalInput").ap()
    wot_d = nc.dram_tensor("wot", [P, OT, O], F16, kind="ExternalInput").ap()
    bias_d = nc.dram_tensor("bias", [P, OT, 3], F32, kind="ExternalInput").ap()
    y_d = nc.dram_tensor("y", [O, N], F32, kind="ExternalOutput").ap()

    with tile.TileContext(nc) as tc:
        with (
            nc.allow_low_precision(reason="fp16/fp8 matmul operands"),
            tc.tile_pool(name="const", bufs=1) as const,
            tc.tile_pool(name="work", bufs=1) as work,
            tc.tile_pool(name="ps", bufs=1, space="PSUM") as ps,
        ):
            # ---- persistent SBUF tensors -------------------------------
            xf3 = const.tile([P, CT, N], F16, tag="xf", name="xf_sb")
            wqkv3 = const.tile([P, CT, 3 * O], F16, tag="wqkv", name="wqkv_sb")
            xf_sb = [xf3[:, c:c + 1, :] for c in range(CT)]
            wqt_sb = [wqkv3[:, c:c + 1, 0:O] for c in range(CT)]
            wkt_sb = [wqkv3[:, c:c + 1, O:2 * O] for c in range(CT)]
            wvt_sb = [wqkv3[:, c:c + 1, 2 * O:3 * O] for c in range(CT)]
            wot3 = const.tile([P, OT, O], F16, tag="wot", name="wot_sb")
            wot_sb = [wot3[:, o:o + 1, :] for o in range(OT)]
            bias3 = const.tile([P, OT, 3], F32, tag="bias", name="bias_sb")
            bq_sb = [bias3[:, o:o + 1, 0:1] for o in range(OT)]
            bk_sb = [bias3[:, o:o + 1, 1:2] for o in range(OT)]
            bo2_sb = [bias3[:, o:o + 1, 2:3] for o in range(OT)]
            # K for score MM j (j=0,1): slot i holds o-tile 2j+i -> fp8 pairs
            k8_sb = [
                const.tile([P, 2, N], FP8, tag=f"k8_{j}", name=f"k8_sb{j}")
                for j in range(2)
            ]
            # V in [o, m] layout, fp16, no bias (bv folded into bo2)
            v16_sb = [
                const.tile([P, N], F16, tag=f"v{o}", name=f"v16_sb{o}")
                for o in range(OT)
            ]
            # (Wo@V)^T m-pair tiles: slot i holds m-tile 2t+i
            wv8_sb = [
                const.tile([P, 2, O], FP8, tag=f"wv{t}", name=f"wv8_sb{t}")
                for t in range(MT2)
            ]
            # all-ones stationary for the denominator partition-reduce MM
            ones_sb = const.tile([P, P], F16, tag="ones", name="ones_sb")
            nc.gpsimd.memset(ones_sb[:], 1.0)
            # fp8 copies of xf and Wk for a DoubleRow K projection (cast
            # on-device by the otherwise-idle DVE during phase 1); c-tiles
            # (0,1) form the DoubleRow pair, c-tile 2 runs as a plain fp8
            # matmul.  Q and V projections stay fp16: fp8 V noise reaches
            # the output ~2x stronger than k noise does via the softmax,
            # and fp8 on BOTH q and k pushes the worst-case error too close
            # to the 2e-2 gate (measured 1.85e-2 in sim vs 1.5e-2 k-only).
            xf8 = const.tile([P, CT, N], FP8, tag="xf8", name="xf8_sb")
            wk8 = const.tile([P, CT, O], FP8, tag="wk8", name="wk8_sb")
            # load order tuned for time-to-first-matmul: Wk slice plus just
            # the first K-proj n-block of xf, then xf in n-block-sized
            # chunks in consumption order; one doorbell per chunk, weights
            # spread over the three DMA-capable queues
            for c in range(CT):
                nc.scalar.dma_start(wqkv3[:, c:c + 1, O:2 * O],
                                    wqkv_d[:, c:c + 1, O:2 * O])
                nc.sync.dma_start(xf3[:, c:c + 1, 0:512],
                                  xf_d[:, c:c + 1, 0:512])
                nc.vector.tensor_copy(wk8[:, c:c + 1, :],
                                      wqkv3[:, c:c + 1, O:2 * O])
                nc.vector.tensor_copy(xf8[:, c:c + 1, 0:512],
                                      xf3[:, c:c + 1, 0:512])
            nc.gpsimd.dma_start(bias3[:], bias_d[:])
            for n0 in (512, 1024):
                nc.sync.dma_start(xf3[:, :, n0:n0 + 512],
                                  xf_d[:, :, n0:n0 + 512])
                nc.vector.tensor_copy(xf8[:, :, n0:n0 + 512],
                                      xf3[:, :, n0:n0 + 512])
            nc.sync.dma_start(xf3[:, :, 1536:N], xf_d[:, :, 1536:N])
            nc.vector.tensor_copy(xf8[:, :, 1536:N], xf3[:, :, 1536:N])
            nc.gpsimd.dma_start(wqkv3[:, :, 2 * O:3 * O],
                                wqkv_d[:, :, 2 * O:3 * O])
            nc.gpsimd.dma_start(wqkv3[:, :, 0:O], wqkv_d[:, :, 0:O])
            nc.scalar.dma_start(wot3[:], wot_d[:])

            # ---- phase 1: K = Wk@xf + bk  (fp8 pair layout [o, m]) -----
            # DoubleRow over c-tiles (0,1) + a plain fp8 matmul for c-tile 2
            for n0, nw in NBLK:
                for o in range(OT):
                    kosl = slice(o * P, (o + 1) * P)
                    kp = ps.tile([P, nw], F32, tag="s", bufs=4, name=f"kp_{o}_{n0}")
                    nc.tensor.matmul(
                        kp[:],
                        wk8[:, 0:2, kosl],
                        xf8[:, 0:2, n0:n0 + nw],
                        start=True,
                        stop=False,
                        perf_mode=DR,
                    )
                    nc.tensor.matmul(
                        kp[:],
                        wk8[:, 2:3, kosl],
                        xf8[:, 2:3, n0:n0 + nw],
                        start=False,
                        stop=True,
                    )
                    nc.scalar.add(
                        k8_sb[o // 2][:, o % 2:o % 2 + 1, n0:n0 + nw],
                        kp[:], bk_sb[o][:],
                    )

            # ---- phase 1b: V = Wv@xf  (fp16, layout [o, m]) ------------
            for n0, nw in NBLK:
                for o in range(OT):
                    osl = slice(o * P, (o + 1) * P)
                    vp = ps.tile([P, nw], F32, tag="s", bufs=4, name=f"vp_{o}_{n0}")
                    for c in range(CT):
                        nc.tensor.matmul(
                            vp[:],
                            wvt_sb[c][:, :, osl],
                            xf_sb[c][:, :, n0:n0 + nw],
                            start=(c == 0),
                            stop=(c == CT - 1),
                        )
                    if o % 2 == 0:
                        nc.scalar.copy(v16_sb[o][:, n0:n0 + nw], vp[:])
                    else:
                        nc.vector.tensor_copy(v16_sb[o][:, n0:n0 + nw], vp[:])

            # ---- phase 1c: (Wo@V)^T = V^T@Wo^T  (fp8 pair layout) ------
            for m in range(MT):
                msl = slice(m * P, (m + 1) * P)
                wp = ps.tile([P, O], F32, tag="s", bufs=4, name=f"wp_{m}")
                for o in range(OT):
                    nc.tensor.matmul(
                        wp[:],
                        v16_sb[o][:, msl],
                        wot_sb[o][:],
                        start=(o == 0),
                        stop=(o == OT - 1),
                    )
                if m % 2 == 0:
                    nc.vector.tensor_copy(
                        wv8_sb[m // 2][:, m % 2:m % 2 + 1, :], wp[:])
                else:
                    nc.scalar.copy(
                        wv8_sb[m // 2][:, m % 2:m % 2 + 1, :], wp[:])

            # ---- phase 2: flash attention over n-blocks ----------------
            # The per-block finish is deferred into the NEXT block's Q
            # section, split in two: the "early" part (denominator + the
            # rb-scale of the output partials, which frees the 4 av PSUM
            # banks) is emitted right after the first Q chunk so the banks
            # are free before the next block's first AV matmul; the "late"
            # part (bias add + store, pure tail work) is emitted after the
            # Q bias-adds so the ACT queue serves q8 first.
            pending_early = None
            pending_late = None
            for n0, nw in NBLK:
                nsl = slice(n0, n0 + nw)
                # Q for this block (fp8 pair layout like K), bias bq added
                q8 = [
                    work.tile([P, 2, nw], FP8, tag=f"q{j}", bufs=3,
                              name=f"q_{n0}_{j}")
                    for j in range(2)
                ]
                for o in range(OT):
                    osl = slice(o * P, (o + 1) * P)
                    qp = ps.tile([P, nw], F32, tag="s", bufs=4, name=f"qp_{n0}_{o}")
                    for c in range(CT):
                        nc.tensor.matmul(
                            qp[:],
                            wqt_sb[c][:, :, osl],
                            xf_sb[c][:, :, nsl],
                            start=(c == 0),
                            stop=(c == CT - 1),
                        )
                    nc.scalar.add(
                        q8[o // 2][:, o % 2:o % 2 + 1, :], qp[:], bq_sb[o][:])
                    if o == 0 and pending_early is not None:
                        pending_early()
                        pending_early = None
                if pending_late is not None:
                    pending_late()
                    pending_late = None

                av_ps = [
                    ps.tile([P, nw], F32, tag=f"av{p}", bufs=1,
                            name=f"av_{n0}_{p}")
                    for p in range(4)
                ]
                # denominator accumulator in the e8 pair layout, fp16: one
                # DVE add per m-pair (DVE ops carry ~100ns+ fixed cost, so
                # fewer wider ops beat many narrow ones)
                eacc = work.tile([P, 2, nw], F16, tag="eacc", bufs=2,
                                 name=f"eacc_{n0}")
                # m-pair tiles: 4 DoubleRow score MMs (contraction 2x256 over
                # o) then 4 DoubleRow AV MMs (contraction 256 over the m-pair)
                for t2 in range(MT2):
                    e8 = work.tile([P, 2, nw], FP8, tag="e", bufs=4,
                                   name=f"e_{n0}_{t2}")
                    for half in (0, 1):
                        m = 2 * t2 + half
                        msl = slice(m * P, (m + 1) * P)
                        sp = ps.tile([P, nw], F32, tag="s", bufs=4,
                                     name=f"sp_{n0}_{m}")
                        nc.tensor.matmul(
                            sp[:],
                            k8_sb[0][:, :, msl],
                            q8[0][:],
                            start=True,
                            stop=False,
                            perf_mode=DR,
                        )
                        nc.tensor.matmul(
                            sp[:],
                            k8_sb[1][:, :, msl],
                            q8[1][:],
                            start=False,
                            stop=True,
                            perf_mode=DR,
                        )
                        nc.scalar.activation(
                            e8[:, half:half + 1, :], sp[:],
                            mybir.ActivationFunctionType.Exp,
                            scale=SCALE,
                        )
                    for p in range(4):
                        psl = slice(p * P, (p + 1) * P)
                        nc.tensor.matmul(
                            av_ps[p][:],
                            wv8_sb[t2][:, :, psl],
                            e8[:],
                            start=(t2 == 0),
                            stop=(t2 == MT2 - 1),
                            perf_mode=DR,
                        )
                    if t2 == 0:
                        nc.vector.tensor_copy(eacc[:], e8[:])
                    else:
                        nc.vector.tensor_add(eacc[:], eacc[:], e8[:])

                def make_finish(n0=n0, nw=nw, nsl=nsl, av_ps=av_ps,
                                eacc=eacc):
                    tmps = []

                    def early():
                        # denominator: fold the pair slots (fp16 in/out runs
                        # at 2x on DVE), then a single all-ones matmul
                        # broadcasts the column sums to every PSUM
                        # partition; fast reciprocal after.
                        ef = work.tile([P, nw], F16, tag="ef", bufs=2,
                                       name=f"ef_{n0}")
                        nc.vector.tensor_add(
                            ef[:], eacc[:, 0:1, :], eacc[:, 1:2, :])
                        dsum = ps.tile([P, nw], F32, tag="s", bufs=4,
                                       name=f"dsum_{n0}")
                        nc.tensor.matmul(dsum[:], ones_sb[:], ef[:],
                                         start=True, stop=True)
                        rb = work.tile([P, nw], F32, tag="rb_sb", bufs=2,
                                       name=f"rb_{n0}")
                        nc.vector.reciprocal_approx_fast(out=rb[:], in_=dsum[:])
                        for p in range(4):
                            tmp = work.tile([P, nw], F32, tag="tmp", bufs=5,
                                            name=f"tmp_{n0}_{p}")
                            nc.vector.tensor_mul(tmp[:], av_ps[p][:], rb[:])
                            tmps.append(tmp)

                    def late(last=False):
                        # bias-add on DVE, not ACT: the ACT queue is
                        # saturated with exps + q8 adds, and a bias-add
                        # placed there delays the next block's exp stream.
                        # For the last block ACT is idle, so use it there.
                        for p in range(4):
                            psl = slice(p * P, (p + 1) * P)
                            outt = work.tile([P, nw], F32, tag="out", bufs=4,
                                             name=f"out_{n0}_{p}")
                            if last:
                                nc.scalar.add(outt[:], tmps[p][:], bo2_sb[p][:])
                            else:
                                nc.vector.tensor_scalar_add(
                                    outt[:], tmps[p][:], bo2_sb[p][:])
                            nc.sync.dma_start(y_d[psl, nsl], outt[:])

                    return early, late

                pending_early, pending_late = make_finish()

            pending_early()
            pending_late(last=True)

    nc.compile()
    return nc


def get_nc():
    if "nc" not in _cache:
        _cache["nc"] = _build_nc()
    return _cache["nc"]


def make_in_maps(x, Wq, bq, Wk, bk, Wv, bv, Wo, bo):
    x = np.asarray(x, np.float32)
    Wq = np.asarray(Wq, np.float32)
    Wk = np.asarray(Wk, np.float32)
    Wv = np.asarray(Wv, np.float32)
    Wo = np.asarray(Wo, np.float32)
    bq = np.asarray(bq, np.float32)
    bk = np.asarray(bk, np.float32)
    bv = np.asarray(bv, np.float32)
    bo = np.asarray(bo, np.float32)

    # channel dims pre-chunked to [128, n_chunks, ...]: one DMA per tensor
    wqkv = np.concatenate([Wq.T, Wk.T, Wv.T], axis=1).astype(np.float16)
    wqkv = wqkv.reshape(CT, P, 3 * O).transpose(1, 0, 2)
    wot = Wo.T.astype(np.float16).reshape(OT, P, O).transpose(1, 0, 2)
    bo2 = (Wo @ bv + bo).astype(np.float32)
    bias = np.stack([bq, bk, bo2], axis=1).astype(np.float32)
    bias = bias.reshape(OT, P, 3).transpose(1, 0, 2)

    xf = x.reshape(B, CT, P, N).transpose(0, 2, 1, 3).astype(np.float16)
    shared = {
        "wqkv": np.ascontiguousarray(wqkv),
        "wot": np.ascontiguousarray(wot),
        "bias": np.ascontiguousarray(bias),
    }
    return [
        {"xf": np.ascontiguousarray(xf[b]), **shared} for b in range(B)
    ]


def kernel(x, Wq, bq, Wk, bk, Wv, bv, Wo, bo):
    from concourse import bass_utils

    nc = get_nc()
    in_maps = make_in_maps(x, Wq, bq, Wk, bk, Wv, bv, Wo, bo)
    res = bass_utils.run_bass_kernel_spmd(nc, in_maps, core_ids=list(range(B)))
    y = np.stack([res.results[b]["y"] for b in range(B)], axis=0)
    return np.ascontiguousarray(y.reshape(B, O, H, W))


# revision 40
# speedup vs baseline: 1.0513x; 1.0513x over previous
"""Trainium2 Bass kernel for CorrelationModule (per-pixel self-attention).

Math (per batch element b, all fp32):
  xf = x[b] reshaped [C=384, N=2304]
  q = Wq@xf + bq, k = Wk@xf + bk, v = Wv@xf + bv       (1x1 convs)
  attn = softmax_m(q^T k / sqrt(512))                  (N x N)
  out = Wo @ (v @ attn^T) + bo                         -> [512, N]

Sharding: batch B=8 data-parallel across the 8 NeuronCores, params replicated.

Per-core kernel layout choices:
  - Scores are computed TRANSPOSED: s_t[m, n] = sum_o k[o,m] q[o,n], so the
    softmax reduction (over m) lands on the PSUM partition axis and is done
    with a ones-vector matmul on the TensorEngine (no 128x128 transposes).
  - exp is taken without max-subtraction: scores*scale ~ N(0, 1/9), so
    exp() cannot overflow for this module's data distribution.
  - Softmax normalization is deferred: AV runs on the unnormalized
    exp-scores; the final tile is multiplied by the broadcast reciprocal
    row sums.  bv is folded into bo' = Wo@bv + bo on the host (valid
    because sum_m attn = 1 after normalization).
  - The output projection is fused into AV by associativity:
    Wo@(V@attn^T) = (Wo@V)@attn^T.  (Wo@V)^T is precomputed once per core
    (72 fp16 matmuls - the same FLOPs the per-block out-projection would
    have cost) and stored as fp8 m-pairs, so the AV DoubleRow matmul
    directly produces output channels and each block finishes with just
    scale+bias+DMA.
  - The two big matmuls (scores K^T Q and AV) run in fp8-e4m3 DoubleRow
    mode: 2 fp8 weights/cell virtualize the PE to a 256-deep contraction,
    2 MACs/cell/cycle.  DoubleRow operands are [128, 2, F] APs - two
    stacked 128-deep k-slices, so producers just write each 128-chunk into
    its slot.  The softmax denominator is summed from the SAME fp8-rounded
    e values the AV matmul consumes, so normalization stays consistent.
    fp8 rounding noise (~2.8%/operand) enters the output through attention
    weights that average 2304 near-uniform terms, shrinking it ~48x; the
    projections stay fp16 where rounding hits the output directly.
  - QKV projections, out-projection: fp16 operands (1 row/cycle + FWL).
    PSUM accumulation stays fp32 everywhere.
  - The softmax denominator partition-reduce is a single all-ones fp16
    matmul (column sums broadcast to every PSUM partition), replacing a
    ~3.5us GpSimd partition_all_reduce on the tail critical path.
  - The av->SBUF copies are emitted inside the deferred finish, AFTER the
    next block's Q bias-adds, so the ACT queue serves q8 (which the next
    score matmul needs in ~2us) before servicing copies the out-projection
    does not need until ~2.6us later.
"""

import numpy as np

B, C, O, H, W = 8, 384, 512, 48, 48
N = H * W  # 2304 tokens
P = 128
CT, OT, MT = C // P, O // P, N // P  # 3, 4, 18
MT2 = MT // 2  # 9 DoubleRow m-pair tiles
NBLK = [(0, 512), (512, 512), (1024, 512), (1536, 512), (2048, 256)]
SCALE = 1.0 / float(np.sqrt(O))

_cache = {}


def _build_nc():
    import concourse.bacc as bacc
    import concourse.tile as tile
    import concourse.mybir as mybir

    F32 = mybir.dt.float32
    F16 = mybir.dt.float16
    FP8 = mybir.dt.float8e4
    DR = mybir.MatmulPerfMode.DoubleRow

    nc = bacc.Bacc(
        "TRN2",
        target_bir_lowering=False,
        debug=False,
        enable_asserts=False,
        num_devices=1,
    )

    # host pre-chunks the channel dims to [128, n_chunks, ...] so each
    # SBUF tensor loads with a single DMA doorbell
    U8 = mybir.dt.uint8
    xf_d = nc.dram_tensor("xf", [P, CT, N], F16, kind="ExternalInput").ap()
    wqkv_d = nc.dram_tensor("wqkv", [P, CT, 3 * O], F16,
                            kind="ExternalInput").ap()
    wot_d = nc.dram_tensor("wot", [P, OT, O], F16, kind="ExternalInput").ap()
    bias_d = nc.dram_tensor("bias", [P, OT, 3], F32, kind="ExternalInput").ap()
    # fp8 operands for the DoubleRow K projection, pre-cast on the host and
    # shipped as raw bytes (uint8) to keep exotic dtypes out of the PJRT
    # path; bitcast to fp8e4 at the point of use
    xf8_d = nc.dram_tensor("xf8", [P, CT, N], U8, kind="ExternalInput").ap()
    wk8_d = nc.dram_tensor("wk8", [P, CT, O], U8, kind="ExternalInput").ap()
    y_d = nc.dram_tensor("y", [O, N], F32, kind="ExternalOutput").ap()

    with tile.TileContext(nc) as tc:
        with (
            nc.allow_low_precision(reason="fp16/fp8 matmul operands"),
            tc.tile_pool(name="const", bufs=1) as const,
            tc.tile_pool(name="work", bufs=1) as work,
            tc.tile_pool(name="ps", bufs=1, space="PSUM") as ps,
        ):
            # ---- persistent SBUF tensors -------------------------------
            xf3 = const.tile([P, CT, N], F16, tag="xf", name="xf_sb")
            wqkv3 = const.tile([P, CT, 3 * O], F16, tag="wqkv", name="wqkv_sb")
            xf_sb = [xf3[:, c:c + 1, :] for c in range(CT)]
            wqt_sb = [wqkv3[:, c:c + 1, 0:O] for c in range(CT)]
            wkt_sb = [wqkv3[:, c:c + 1, O:2 * O] for c in range(CT)]
            wvt_sb = [wqkv3[:, c:c + 1, 2 * O:3 * O] for c in range(CT)]
            wot3 = const.tile([P, OT, O], F16, tag="wot", name="wot_sb")
            wot_sb = [wot3[:, o:o + 1, :] for o in range(OT)]
            bias3 = const.tile([P, OT, 3], F32, tag="bias", name="bias_sb")
            bq_sb = [bias3[:, o:o + 1, 0:1] for o in range(OT)]
            bk_sb = [bias3[:, o:o + 1, 1:2] for o in range(OT)]
            bo2_sb = [bias3[:, o:o + 1, 2:3] for o in range(OT)]
            # K for score MM j (j=0,1): slot i holds o-tile 2j+i -> fp8 pairs
            k8_sb = [
                const.tile([P, 2, N], FP8, tag=f"k8_{j}", name=f"k8_sb{j}")
                for j in range(2)
            ]
            # V in [o, m] layout, fp16, no bias (bv folded into bo2)
            v16_sb = [
                const.tile([P, N], F16, tag=f"v{o}", name=f"v16_sb{o}")
                for o in range(OT)
            ]
            # (Wo@V)^T m-pair tiles: slot i holds m-tile 2t+i
            wv8_sb = [
                const.tile([P, 2, O], FP8, tag=f"wv{t}", name=f"wv8_sb{t}")
                for t in range(MT2)
            ]
            # all-ones stationary for the denominator partition-reduce MM
            ones_sb = const.tile([P, P], F16, tag="ones", name="ones_sb")
            nc.gpsimd.memset(ones_sb[:], 1.0)
            # fp8 operands for the DoubleRow K projection; c-tiles (0,1)
            # form the DoubleRow pair, c-tile 2 runs as a plain fp8 matmul.
            # Q and V projections stay fp16: fp8 V noise reaches the output
            # ~2x stronger than k noise does via the softmax, and fp8 on
            # BOTH q and k pushes the worst-case error too close to the
            # 2e-2 gate (measured 1.85e-2 in sim vs 1.61e-2 k-only).
            xf8 = const.tile([P, CT, N], FP8, tag="xf8", name="xf8_sb")
            wk8 = const.tile([P, CT, O], FP8, tag="wk8", name="wk8_sb")
            # load order tuned for time-to-first-matmul: the fp8 K operands
            # first (196KB each), then xf8/xf16 in n-block-sized chunks in
            # consumption order; one doorbell per chunk, weights spread
            # over the three DMA-capable queues.  The fp16 Wk columns are
            # never loaded - only the fp8 copy is used.
            nc.scalar.dma_start(wk8[:], wk8_d[:].bitcast(FP8))
            nc.sync.dma_start(xf8[:, :, 0:512],
                              xf8_d[:, :, 0:512].bitcast(FP8))
            nc.gpsimd.dma_start(bias3[:], bias_d[:])
            for n0, w in ((512, 512), (1024, 512), (1536, 768)):
                nc.sync.dma_start(xf8[:, :, n0:n0 + w],
                                  xf8_d[:, :, n0:n0 + w].bitcast(FP8))
            for n0, w in ((0, 512), (512, 512), (1024, 512), (1536, 768)):
                nc.sync.dma_start(xf3[:, :, n0:n0 + w],
                                  xf_d[:, :, n0:n0 + w])
            nc.scalar.dma_start(wqkv3[:, :, 2 * O:3 * O],
                                wqkv_d[:, :, 2 * O:3 * O])
            nc.scalar.dma_start(wot3[:], wot_d[:])
            nc.gpsimd.dma_start(wqkv3[:, :, 0:O], wqkv_d[:, :, 0:O])

            # ---- phase 1: K = Wk@xf + bk  (fp8 pair layout [o, m]) -----
            # DoubleRow over c-tiles (0,1) + a plain fp8 matmul for c-tile 2
            for n0, nw in NBLK:
                for o in range(OT):
                    kosl = slice(o * P, (o + 1) * P)
                    kp = ps.tile([P, nw], F32, tag="s", bufs=4, name=f"kp_{o}_{n0}")
                    nc.tensor.matmul(
                        kp[:],
                        wk8[:, 0:2, kosl],
                        xf8[:, 0:2, n0:n0 + nw],
                        start=True,
                        stop=False,
                        perf_mode=DR,
                    )
                    nc.tensor.matmul(
                        kp[:],
                        wk8[:, 2:3, kosl],
                        xf8[:, 2:3, n0:n0 + nw],
                        start=False,
                        stop=True,
                    )
                    nc.scalar.add(
                        k8_sb[o // 2][:, o % 2:o % 2 + 1, n0:n0 + nw],
                        kp[:], bk_sb[o][:],
                    )

            # ---- phase 1b: V = Wv@xf  (fp16, layout [o, m]) ------------
            for n0, nw in NBLK:
                for o in range(OT):
                    osl = slice(o * P, (o + 1) * P)
                    vp = ps.tile([P, nw], F32, tag="s", bufs=4, name=f"vp_{o}_{n0}")
                    for c in range(CT):
                        nc.tensor.matmul(
                            vp[:],
                            wvt_sb[c][:, :, osl],
                            xf_sb[c][:, :, n0:n0 + nw],
                            start=(c == 0),
                            stop=(c == CT - 1),
                        )
                    if o % 2 == 0:
                        nc.scalar.copy(v16_sb[o][:, n0:n0 + nw], vp[:])
                    else:
                        nc.vector.tensor_copy(v16_sb[o][:, n0:n0 + nw], vp[:])

            # ---- phase 1c: (Wo@V)^T = V^T@Wo^T  (fp8 pair layout) ------
            for m in range(MT):
                msl = slice(m * P, (m + 1) * P)
                wp = ps.tile([P, O], F32, tag="s", bufs=4, name=f"wp_{m}")
                for o in range(OT):
                    nc.tensor.matmul(
                        wp[:],
                        v16_sb[o][:, msl],
                        wot_sb[o][:],
                        start=(o == 0),
                        stop=(o == OT - 1),
                    )
                if m % 2 == 0:
                    nc.vector.tensor_copy(
                        wv8_sb[m // 2][:, m % 2:m % 2 + 1, :], wp[:])
                else:
                    nc.scalar.copy(
                        wv8_sb[m // 2][:, m % 2:m % 2 + 1, :], wp[:])

            # ---- phase 2: flash attention over n-blocks ----------------
            # The per-block finish is deferred into the NEXT block's Q
            # section, split in two: the "early" part (denominator + the
            # rb-scale of the output partials, which frees the 4 av PSUM
            # banks) is emitted right after the first Q chunk so the banks
            # are free before the next block's first AV matmul; the "late"
            # part (bias add + store, pure tail work) is emitted after the
            # Q bias-adds so the ACT queue serves q8 first.
            pending_early = None
            pending_late = None
            for n0, nw in NBLK:
                nsl = slice(n0, n0 + nw)
                # Q for this block (fp8 pair layout like K), bias bq added
                q8 = [
                    work.tile([P, 2, nw], FP8, tag=f"q{j}", bufs=3,
                              name=f"q_{n0}_{j}")
                    for j in range(2)
                ]
                for o in range(OT):
                    osl = slice(o * P, (o + 1) * P)
                    qp = ps.tile([P, nw], F32, tag="s", bufs=4, name=f"qp_{n0}_{o}")
                    for c in range(CT):
                        nc.tensor.matmul(
                            qp[:],
                            wqt_sb[c][:, :, osl],
                            xf_sb[c][:, :, nsl],
                            start=(c == 0),
                            stop=(c == CT - 1),
                        )
                    nc.scalar.add(
                        q8[o // 2][:, o % 2:o % 2 + 1, :], qp[:], bq_sb[o][:])
                    if o == 0 and pending_early is not None:
                        pending_early()
                        pending_early = None
                if pending_late is not None:
                    pending_late()
                    pending_late = None

                av_ps = [
                    ps.tile([P, nw], F32, tag=f"av{p}", bufs=1,
                            name=f"av_{n0}_{p}")
                    for p in range(4)
                ]
                # denominator accumulator in the e8 pair layout, fp16: one
                # DVE add per m-pair (DVE ops carry ~100ns+ fixed cost, so
                # fewer wider ops beat many narrow ones)
                eacc = work.tile([P, 2, nw], F16, tag="eacc", bufs=2,
                                 name=f"eacc_{n0}")
                # m-pair tiles: 4 DoubleRow score MMs (contraction 2x256 over
                # o) then 4 DoubleRow AV MMs (contraction 256 over the m-pair)
                for t2 in range(MT2):
                    e8 = work.tile([P, 2, nw], FP8, tag="e", bufs=4,
                                   name=f"e_{n0}_{t2}")
                    for half in (0, 1):
                        m = 2 * t2 + half
                        msl = slice(m * P, (m + 1) * P)
                        sp = ps.tile([P, nw], F32, tag="s", bufs=4,
                                     name=f"sp_{n0}_{m}")
                        nc.tensor.matmul(
                            sp[:],
                            k8_sb[0][:, :, msl],
                            q8[0][:],
                            start=True,
                            stop=False,
                            perf_mode=DR,
                        )
                        nc.tensor.matmul(
                            sp[:],
                            k8_sb[1][:, :, msl],
                            q8[1][:],
                            start=False,
                            stop=True,
                            perf_mode=DR,
                        )
                        nc.scalar.activation(
                            e8[:, half:half + 1, :], sp[:],
                            mybir.ActivationFunctionType.Exp,
                            scale=SCALE,
                        )
                    for p in range(4):
                        psl = slice(p * P, (p + 1) * P)
                        nc.tensor.matmul(
                            av_ps[p][:],
                            wv8_sb[t2][:, :, psl],
                            e8[:],
                            start=(t2 == 0),
                            stop=(t2 == MT2 - 1),
                            perf_mode=DR,
                        )
                    if t2 == 0:
                        nc.vector.tensor_copy(eacc[:], e8[:])
                    else:
                        nc.vector.tensor_add(eacc[:], eacc[:], e8[:])

                def make_finish(n0=n0, nw=nw, nsl=nsl, av_ps=av_ps,
                                eacc=eacc):
                    tmps = []

                    def early():
                        # denominator: fold the pair slots (fp16 in/out runs
                        # at 2x on DVE), then a single all-ones matmul
                        # broadcasts the column sums to every PSUM
                        # partition; fast reciprocal after.
                        ef = work.tile([P, nw], F16, tag="ef", bufs=2,
                                       name=f"ef_{n0}")
                        nc.vector.tensor_add(
                            ef[:], eacc[:, 0:1, :], eacc[:, 1:2, :])
                        dsum = ps.tile([P, nw], F32, tag="s", bufs=4,
                                       name=f"dsum_{n0}")
                        nc.tensor.matmul(dsum[:], ones_sb[:], ef[:],
                                         start=True, stop=True)
                        rb = work.tile([P, nw], F32, tag="rb_sb", bufs=2,
                                       name=f"rb_{n0}")
                        nc.vector.reciprocal_approx_fast(out=rb[:], in_=dsum[:])
                        for p in range(4):
                            tmp = work.tile([P, nw], F32, tag="tmp", bufs=5,
                                            name=f"tmp_{n0}_{p}")
                            nc.vector.tensor_mul(tmp[:], av_ps[p][:], rb[:])
                            tmps.append(tmp)

                    def late(last=False):
                        # bias-add on DVE, not ACT: the ACT queue is
                        # saturated with exps + q8 adds, and a bias-add
                        # placed there delays the next block's exp stream.
                        # For the last block ACT is idle, so use it there.
                        for p in range(4):
                            psl = slice(p * P, (p + 1) * P)
                            outt = work.tile([P, nw], F32, tag="out", bufs=4,
                                             name=f"out_{n0}_{p}")
                            if last:
                                nc.scalar.add(outt[:], tmps[p][:], bo2_sb[p][:])
                            else:
                                nc.vector.tensor_scalar_add(
                                    outt[:], tmps[p][:], bo2_sb[p][:])
                            nc.sync.dma_start(y_d[psl, nsl], outt[:])

                    return early, late

                pending_early, pending_late = make_finish()

            pending_early()
            pending_late(last=True)

    nc.compile()
    return nc


def get_nc():
    if "nc" not in _cache:
        _cache["nc"] = _build_nc()
    return _cache["nc"]


def make_in_maps(x, Wq, bq, Wk, bk, Wv, bv, Wo, bo):
    x = np.asarray(x, np.float32)
    Wq = np.asarray(Wq, np.float32)
    Wk = np.asarray(Wk, np.float32)
    Wv = np.asarray(Wv, np.float32)
    Wo = np.asarray(Wo, np.float32)
    bq = np.asarray(bq, np.float32)
    bk = np.asarray(bk, np.float32)
    bv = np.asarray(bv, np.float32)
    bo = np.asarray(bo, np.float32)

    # channel dims pre-chunked to [128, n_chunks, ...]: one DMA per tensor
    wqkv = np.concatenate([Wq.T, Wk.T, Wv.T], axis=1).astype(np.float16)
    wqkv = wqkv.reshape(CT, P, 3 * O).transpose(1, 0, 2)
    wot = Wo.T.astype(np.float16).reshape(OT, P, O).transpose(1, 0, 2)
    bo2 = (Wo @ bv + bo).astype(np.float32)
    bias = np.stack([bq, bk, bo2], axis=1).astype(np.float32)
    bias = bias.reshape(OT, P, 3).transpose(1, 0, 2)

    import ml_dtypes

    fp8 = ml_dtypes.float8_e4m3  # trn float8e4: same bit layout
    xf = x.reshape(B, CT, P, N).transpose(0, 2, 1, 3).astype(np.float16)
    xf8 = np.ascontiguousarray(xf).astype(fp8).view(np.uint8)
    wk8 = np.ascontiguousarray(
        Wk.T.reshape(CT, P, O).transpose(1, 0, 2)).astype(fp8).view(np.uint8)
    shared = {
        "wqkv": np.ascontiguousarray(wqkv),
        "wot": np.ascontiguousarray(wot),
        "bias": np.ascontiguousarray(bias),
        "wk8": wk8,
    }
    return [
        {"xf": np.ascontiguousarray(xf[b]), "xf8": xf8[b], **shared}
        for b in range(B)
    ]


def kernel(x, Wq, bq, Wk, bk, Wv, bv, Wo, bo):
    from concourse import bass_utils

    nc = get_nc()
    in_maps = make_in_maps(x, Wq, bq, Wk, bk, Wv, bv, Wo, bo)
    res = bass_utils.run_bass_kernel_spmd(nc, in_maps, core_ids=list(range(B)))
    y = np.stack([res.results[b]["y"] for b in range(B)], axis=0)
    return np.ascontiguousarray(y.reshape(B, O, H, W))


# revision 49
# speedup vs baseline: 1.1538x; 1.0975x over previous
"""Trainium2 Bass kernel for CorrelationModule (per-pixel self-attention).

Math (per batch element b, all fp32):
  xf = x[b] reshaped [C=384, N=2304]
  q = Wq@xf + bq, k = Wk@xf + bk, v = Wv@xf + bv       (1x1 convs)
  attn = softmax_m(q^T k / sqrt(512))                  (N x N)
  out = Wo @ (v @ attn^T) + bo                         -> [512, N]

Sharding: batch B=8 data-parallel across the 8 NeuronCores, params replicated.

Per-core kernel layout choices:
  - Scores are computed TRANSPOSED: s_t[m, n] = sum_o k[o,m] q[o,n], so the
    softmax reduction (over m) lands on the PSUM partition axis and is done
    with a ones-vector matmul on the TensorEngine (no 128x128 transposes).
  - exp is taken without max-subtraction: scores*scale ~ N(0, 1/9), so
    exp() cannot overflow for this module's data distribution.
  - Softmax normalization is deferred: AV runs on the unnormalized
    exp-scores; the final tile is multiplied by the broadcast reciprocal
    row sums.  bv is folded into bo' = Wo@bv + bo on the host (valid
    because sum_m attn = 1 after normalization).
  - The output projection is fused into AV by associativity:
    Wo@(V@attn^T) = (Wo@V)@attn^T.  (Wo@V)^T is precomputed once per core
    (72 fp16 matmuls - the same FLOPs the per-block out-projection would
    have cost) and stored as fp8 m-pairs, so the AV DoubleRow matmul
    directly produces output channels and each block finishes with just
    scale+bias+DMA.
  - The two big matmuls (scores K^T Q and AV) run in fp8-e4m3 DoubleRow
    mode: 2 fp8 weights/cell virtualize the PE to a 256-deep contraction,
    2 MACs/cell/cycle.  DoubleRow operands are [128, 2, F] APs - two
    stacked 128-deep k-slices, so producers just write each 128-chunk into
    its slot.  The softmax denominator is summed from the SAME fp8-rounded
    e values the AV matmul consumes, so normalization stays consistent.
    fp8 rounding noise (~2.8%/operand) enters the output through attention
    weights that average 2304 near-uniform terms, shrinking it ~48x; the
    projections stay fp16 where rounding hits the output directly.
  - QKV projections, out-projection: fp16 operands (1 row/cycle + FWL).
    PSUM accumulation stays fp32 everywhere.
  - The softmax denominator partition-reduce is a single all-ones fp16
    matmul (column sums broadcast to every PSUM partition), replacing a
    ~3.5us GpSimd partition_all_reduce on the tail critical path.
  - The av->SBUF copies are emitted inside the deferred finish, AFTER the
    next block's Q bias-adds, so the ACT queue serves q8 (which the next
    score matmul needs in ~2us) before servicing copies the out-projection
    does not need until ~2.6us later.
"""

import numpy as np

B, C, O, H, W = 8, 384, 512, 48, 48
N = H * W  # 2304 tokens
P = 128
CT, OT, MT = C // P, O // P, N // P  # 3, 4, 18
MT2 = MT // 2  # 9 DoubleRow m-pair tiles
NBLK = [(0, 512), (512, 512), (1024, 512), (1536, 512), (2048, 256)]
SCALE = 1.0 / float(np.sqrt(O))

_cache = {}


def _build_nc():
    import concourse.bacc as bacc
    import concourse.tile as tile
    import concourse.mybir as mybir

    F32 = mybir.dt.float32
    F16 = mybir.dt.float16
    FP8 = mybir.dt.float8e4
    DR = mybir.MatmulPerfMode.DoubleRow

    nc = bacc.Bacc(
        "TRN2",
        target_bir_lowering=False,
        debug=False,
        enable_asserts=False,
        num_devices=1,
    )

    # host pre-chunks the channel dims to [128, n_chunks, ...] so each
    # SBUF tensor loads with a single DMA doorbell
    U8 = mybir.dt.uint8
    xf_d = nc.dram_tensor("xf", [P, CT, N], F16, kind="ExternalInput").ap()
    wq_d = nc.dram_tensor("wq", [P, CT, O], F16, kind="ExternalInput").ap()
    wovvt_d = nc.dram_tensor("wovvt", [P, CT, O], F16,
                             kind="ExternalInput").ap()
    bias_d = nc.dram_tensor("bias", [P, OT, 3], F32, kind="ExternalInput").ap()
    # fp8 operands for the DoubleRow K projection, pre-cast on the host and
    # shipped as raw bytes (uint8) to keep exotic dtypes out of the PJRT
    # path; bitcast to fp8e4 at the point of use
    xf8_d = nc.dram_tensor("xf8", [P, CT, N], U8, kind="ExternalInput").ap()
    wk8_d = nc.dram_tensor("wk8", [P, CT, O], U8, kind="ExternalInput").ap()
    # y stored [partition, o-chunk, col] so each block writes one DMA;
    # the host transposes back to [O, N]
    y_d = nc.dram_tensor("y", [P, OT, N], F32, kind="ExternalOutput").ap()

    with tile.TileContext(nc) as tc:
        with (
            nc.allow_low_precision(reason="fp16/fp8 matmul operands"),
            tc.tile_pool(name="const", bufs=1) as const,
            tc.tile_pool(name="work", bufs=1) as work,
            tc.tile_pool(name="ps", bufs=1, space="PSUM") as ps,
        ):
            # ---- persistent SBUF tensors -------------------------------
            xf3 = const.tile([P, CT, N], F16, tag="xf", name="xf_sb")
            wq3 = const.tile([P, CT, O], F16, tag="wq", name="wq_sb")
            wovvt3 = const.tile([P, CT, O], F16, tag="wovvt", name="wovvt_sb")
            xf_sb = [xf3[:, c:c + 1, :] for c in range(CT)]
            wqt_sb = [wq3[:, c:c + 1, :] for c in range(CT)]
            bias3 = const.tile([P, OT, 3], F32, tag="bias", name="bias_sb")
            bq_sb = [bias3[:, o:o + 1, 0:1] for o in range(OT)]
            bk_sb = [bias3[:, o:o + 1, 1:2] for o in range(OT)]
            bo2_sb = [bias3[:, o:o + 1, 2:3] for o in range(OT)]
            # K for score MM j (j=0,1): slot i holds o-tile 2j+i -> fp8 pairs
            k8_sb = [
                const.tile([P, 2, N], FP8, tag=f"k8_{j}", name=f"k8_sb{j}")
                for j in range(2)
            ]
            # (Wo@V)^T m-pair tiles: slot i holds m-tile 2t+i
            wv8_sb = [
                const.tile([P, 2, O], FP8, tag=f"wv{t}", name=f"wv8_sb{t}")
                for t in range(MT2)
            ]
            # all-ones stationary for the denominator partition-reduce MM
            ones_sb = const.tile([P, P], F16, tag="ones", name="ones_sb")
            nc.gpsimd.memset(ones_sb[:], 1.0)
            # fp8 operands for the DoubleRow K projection; c-tiles (0,1)
            # form the DoubleRow pair, c-tile 2 runs as a plain fp8 matmul.
            # Q and V projections stay fp16: fp8 V noise reaches the output
            # ~2x stronger than k noise does via the softmax, and fp8 on
            # BOTH q and k pushes the worst-case error too close to the
            # 2e-2 gate (measured 1.85e-2 in sim vs 1.61e-2 k-only).
            xf8 = const.tile([P, CT, N], FP8, tag="xf8", name="xf8_sb")
            wk8 = const.tile([P, CT, O], FP8, tag="wk8", name="wk8_sb")
            # load order tuned for time-to-first-matmul: the fp8 K operands
            # first (196KB each), then xf8/xf16 in n-block-sized chunks in
            # consumption order; one doorbell per chunk, weights spread
            # over the three DMA-capable queues.  The fp16 Wk columns are
            # never loaded - only the fp8 copy is used.
            nc.scalar.dma_start(wk8[:, :, 0:P], wk8_d[:, :, 0:P].bitcast(FP8))
            nc.sync.dma_start(xf8[:, 0:2, 0:512],
                              xf8_d[:, 0:2, 0:512].bitcast(FP8))
            nc.gpsimd.dma_start(bias3[:], bias_d[:])
            nc.gpsimd.dma_start(xf8[:, 2:3, 0:512],
                                xf8_d[:, 2:3, 0:512].bitcast(FP8))
            nc.scalar.dma_start(wk8[:, :, P:O], wk8_d[:, :, P:O].bitcast(FP8))
            for q, (n0, w) in zip(
                    (nc.sync, nc.gpsimd, nc.sync),
                    ((512, 512), (1024, 512), (1536, 768))):
                q.dma_start(xf8[:, :, n0:n0 + w],
                            xf8_d[:, :, n0:n0 + w].bitcast(FP8))
            nc.scalar.dma_start(wovvt3[:], wovvt_d[:])
            for q, (n0, w) in zip(
                    (nc.gpsimd, nc.sync, nc.gpsimd, nc.sync),
                    ((0, 512), (512, 512), (1024, 512), (1536, 768))):
                q.dma_start(xf3[:, :, n0:n0 + w], xf_d[:, :, n0:n0 + w])
            nc.scalar.dma_start(wq3[:], wq_d[:])

            # ---- phase 1: K = Wk@xf + bk  (fp8 pair layout [o, m]) -----
            # DoubleRow over c-tiles (0,1) + a plain fp8 matmul for c-tile 2
            for n0, nw in NBLK:
                for o in range(OT):
                    kosl = slice(o * P, (o + 1) * P)
                    kp = ps.tile([P, nw], F32, tag="s", bufs=4, name=f"kp_{o}_{n0}")
                    nc.tensor.matmul(
                        kp[:],
                        wk8[:, 0:2, kosl],
                        xf8[:, 0:2, n0:n0 + nw],
                        start=True,
                        stop=False,
                        perf_mode=DR,
                    )
                    nc.tensor.matmul(
                        kp[:],
                        wk8[:, 2:3, kosl],
                        xf8[:, 2:3, n0:n0 + nw],
                        start=False,
                        stop=True,
                    )
                    nc.scalar.add(
                        k8_sb[o // 2][:, o % 2:o % 2 + 1, n0:n0 + nw],
                        kp[:], bk_sb[o][:],
                    )

            # ---- phase 1b: (Wo@V)^T = xf^T @ (Wo@Wv)^T  (fp8 pairs) ----
            # Wo@Wv is folded on the host, so one 384-contraction per
            # m-tile produces the output-space values directly
            for m in range(MT):
                msl = slice(m * P, (m + 1) * P)
                wp = ps.tile([P, O], F32, tag="s", bufs=4, name=f"wp_{m}")
                for c in range(CT):
                    nc.tensor.matmul(
                        wp[:],
                        xf3[:, c:c + 1, msl],
                        wovvt3[:, c:c + 1, :],
                        start=(c == 0),
                        stop=(c == CT - 1),
                    )
                if m % 2 == 0:
                    nc.vector.tensor_copy(
                        wv8_sb[m // 2][:, m % 2:m % 2 + 1, :], wp[:])
                else:
                    nc.scalar.copy(
                        wv8_sb[m // 2][:, m % 2:m % 2 + 1, :], wp[:])

            # ---- phase 2: flash attention over n-blocks ----------------
            # The per-block finish is deferred into the NEXT block's Q
            # section, split in two: the "early" part (denominator + the
            # rb-scale of the output partials, which frees the 4 av PSUM
            # banks) is emitted right after the first Q chunk so the banks
            # are free before the next block's first AV matmul; the "late"
            # part (bias add + store, pure tail work) is emitted after the
            # Q bias-adds so the ACT queue serves q8 first.
            pending_early = None
            pending_late = None
            for n0, nw in NBLK:
                nsl = slice(n0, n0 + nw)
                # Q for this block (fp8 pair layout like K), bias bq added
                q8 = [
                    work.tile([P, 2, nw], FP8, tag=f"q{j}", bufs=3,
                              name=f"q_{n0}_{j}")
                    for j in range(2)
                ]
                for o in range(OT):
                    osl = slice(o * P, (o + 1) * P)
                    qp = ps.tile([P, nw], F32, tag="s", bufs=4, name=f"qp_{n0}_{o}")
                    for c in range(CT):
                        nc.tensor.matmul(
                            qp[:],
                            wqt_sb[c][:, :, osl],
                            xf_sb[c][:, :, nsl],
                            start=(c == 0),
                            stop=(c == CT - 1),
                        )
                    nc.scalar.add(
                        q8[o // 2][:, o % 2:o % 2 + 1, :], qp[:], bq_sb[o][:])
                    if o == 0 and pending_early is not None:
                        pending_early()
                        pending_early = None
                if pending_late is not None:
                    pending_late()
                    pending_late = None

                av_ps = [
                    ps.tile([P, nw], F32, tag=f"av{p}", bufs=1,
                            name=f"av_{n0}_{p}")
                    for p in range(4)
                ]
                # denominator accumulator in the e8 pair layout, fp16: one
                # DVE add per m-pair (DVE ops carry ~100ns+ fixed cost, so
                # fewer wider ops beat many narrow ones)
                eacc = work.tile([P, 2, nw], F16, tag="eacc", bufs=2,
                                 name=f"eacc_{n0}")
                # m-pair tiles: 4 DoubleRow score MMs (contraction 2x256 over
                # o) then 4 DoubleRow AV MMs (contraction 256 over the m-pair)
                for t2 in range(MT2):
                    e8 = work.tile([P, 2, nw], FP8, tag="e", bufs=4,
                                   name=f"e_{n0}_{t2}")
                    for half in (0, 1):
                        m = 2 * t2 + half
                        msl = slice(m * P, (m + 1) * P)
                        sp = ps.tile([P, nw], F32, tag="s", bufs=4,
                                     name=f"sp_{n0}_{m}")
                        nc.tensor.matmul(
                            sp[:],
                            k8_sb[0][:, :, msl],
                            q8[0][:],
                            start=True,
                            stop=False,
                            perf_mode=DR,
                        )
                        nc.tensor.matmul(
                            sp[:],
                            k8_sb[1][:, :, msl],
                            q8[1][:],
                            start=False,
                            stop=True,
                            perf_mode=DR,
                        )
                        nc.scalar.activation(
                            e8[:, half:half + 1, :], sp[:],
                            mybir.ActivationFunctionType.Exp,
                            scale=SCALE,
                        )
                    for p in range(4):
                        psl = slice(p * P, (p + 1) * P)
                        nc.tensor.matmul(
                            av_ps[p][:],
                            wv8_sb[t2][:, :, psl],
                            e8[:],
                            start=(t2 == 0),
                            stop=(t2 == MT2 - 1),
                            perf_mode=DR,
                        )
                    if t2 == 0:
                        nc.vector.tensor_copy(eacc[:], e8[:])
                    else:
                        nc.vector.tensor_add(eacc[:], eacc[:], e8[:])

                def make_finish(n0=n0, nw=nw, nsl=nsl, av_ps=av_ps,
                                eacc=eacc):
                    tmps = []

                    def early():
                        # denominator: fold the pair slots (fp16 in/out runs
                        # at 2x on DVE), then a single all-ones matmul
                        # broadcasts the column sums to every PSUM
                        # partition; fast reciprocal after.
                        ef = work.tile([P, nw], F16, tag="ef", bufs=2,
                                       name=f"ef_{n0}")
                        nc.vector.tensor_add(
                            ef[:], eacc[:, 0:1, :], eacc[:, 1:2, :])
                        dsum = ps.tile([P, nw], F32, tag="s", bufs=4,
                                       name=f"dsum_{n0}")
                        nc.tensor.matmul(dsum[:], ones_sb[:], ef[:],
                                         start=True, stop=True)
                        rb = work.tile([P, nw], F32, tag="rb_sb", bufs=2,
                                       name=f"rb_{n0}")
                        nc.vector.reciprocal_approx_fast(out=rb[:], in_=dsum[:])
                        tmp = work.tile([P, OT, nw], F32, tag="tmp", bufs=2,
                                        name=f"tmp_{n0}")
                        for p in range(4):
                            nc.vector.tensor_mul(
                                tmp[:, p:p + 1, :], av_ps[p][:], rb[:])
                        tmps.append(tmp)

                    def late(last=False):
                        # bias-add on DVE, not ACT: the ACT queue is
                        # saturated with exps + q8 adds, and a bias-add
                        # placed there delays the next block's exp stream.
                        # For the last block ACT is idle, so use it there.
                        # One y DMA for the whole block (y is stored
                        # [partition, o-chunk, col]).
                        outt = work.tile([P, OT, nw], F32, tag="out", bufs=2,
                                         name=f"out_{n0}")
                        for p in range(4):
                            if last:
                                nc.scalar.add(outt[:, p:p + 1, :],
                                              tmps[0][:, p:p + 1, :],
                                              bo2_sb[p][:])
                            else:
                                nc.vector.tensor_scalar_add(
                                    outt[:, p:p + 1, :],
                                    tmps[0][:, p:p + 1, :], bo2_sb[p][:])
                        nc.sync.dma_start(y_d[:, :, nsl], outt[:])

                    return early, late

                pending_early, pending_late = make_finish()

            pending_early()
            pending_late(last=True)

    nc.compile()
    return nc


def get_nc():
    if "nc" not in _cache:
        _cache["nc"] = _build_nc()
    return _cache["nc"]


def make_in_maps(x, Wq, bq, Wk, bk, Wv, bv, Wo, bo):
    x = np.asarray(x, np.float32)
    Wq = np.asarray(Wq, np.float32)
    Wk = np.asarray(Wk, np.float32)
    Wv = np.asarray(Wv, np.float32)
    Wo = np.asarray(Wo, np.float32)
    bq = np.asarray(bq, np.float32)
    bk = np.asarray(bk, np.float32)
    bv = np.asarray(bv, np.float32)
    bo = np.asarray(bo, np.float32)

    # channel dims pre-chunked to [128, n_chunks, ...]: one DMA per tensor
    bo2 = (Wo @ bv + bo).astype(np.float32)
    bias = np.stack([bq, bk, bo2], axis=1).astype(np.float32)
    bias = bias.reshape(OT, P, 3).transpose(1, 0, 2)

    import ml_dtypes

    fp8 = ml_dtypes.float8_e4m3  # trn float8e4: same bit layout
    xf = x.reshape(B, CT, P, N).transpose(0, 2, 1, 3).astype(np.float16)
    xf8 = np.ascontiguousarray(xf).astype(fp8).view(np.uint8)
    wk8 = np.ascontiguousarray(
        Wk.T.reshape(CT, P, O).transpose(1, 0, 2)).astype(fp8).view(np.uint8)
    wq16 = np.ascontiguousarray(
        Wq.T.reshape(CT, P, O).transpose(1, 0, 2)).astype(np.float16)
    # out-projection folded into V on the host: Wo@(Wv@xf) = (Wo@Wv)@xf
    wovvt = np.ascontiguousarray(
        (Wo @ Wv).T.reshape(CT, P, O).transpose(1, 0, 2)).astype(np.float16)
    shared = {
        "wq": wq16,
        "wovvt": wovvt,
        "bias": np.ascontiguousarray(bias),
        "wk8": wk8,
    }
    return [
        {"xf": np.ascontiguousarray(xf[b]), "xf8": xf8[b], **shared}
        for b in range(B)
    ]


def fix_y(arr):
    """Device y layout is [128, 4, N] (partition, o-chunk, col) so each
    block stores with one DMA; restore the logical [O, N]."""
    return arr.reshape(P, OT, N).transpose(1, 0, 2).reshape(O, N)


def kernel(x, Wq, bq, Wk, bk, Wv, bv, Wo, bo):
    from concourse import bass_utils

    nc = get_nc()
    in_maps = make_in_maps(x, Wq, bq, Wk, bk, Wv, bv, Wo, bo)
    res = bass_utils.run_bass_kernel_spmd(nc, in_maps, core_ids=list(range(B)))
    y = np.stack([fix_y(res.results[b]["y"]) for b in range(B)], axis=0)
    return np.ascontiguousarray(y.reshape(B, O, H, W))
